# revision 1
# baseline (speedup 1.0000x reference)
"""Linear attention kernel for 8 Trainium2 NeuronCores.

Sharding: core = 2*b + hg  (b in 0..3 batches, hg in 0..1 head-groups of 8 heads).
Fully data-parallel — no collectives; host sums the two head-group partials per
batch (f32) and adds the bias.

Per-core math (T=4096 tokens, CH=512 = 8 heads x 64, DIM=1024):
  Phase 1 (per 512-token block): k,v = x @ Wk, x @ Wv token-major; elu+1 on k;
    qT = (x @ Wq)^T c-major with elu+1 (persisted for phase 2);
    kvT += v^T k per head-pair (diagonal 64-blocks), z += ones^T k, in PSUM.
    The z/kv matmuls are deferred one token-chunk so the in-order PE never
    waits on the elu chain.
  Boundary: evict kvT/z; M = kvT^T @ W2 interleaved with the first den
    stages; Zb = block-diag expansion of z.
  Phase 2 (per 512-token block, software-pipelined 4 blocks deep so the
    in-order PE never waits on the r chain): den = Zb^T qT [8, 512];
    r = 1/(den+1e-6) computed in a [128,32] partition-blocked layout (DMA
    reshape both ways; 16x fewer elems per DVE lane than [8,512]);
    rbc = E^T r broadcast matmul; qsc = qT * rbc;
    y = qsc^T @ M, written bf16 (host accumulates in f32).

  A short burst of dummy matmuls at the start lifts the PE HAM clock gate
  (1.2 -> 2.4 GHz) while the first input DMAs are still in flight.
"""

import sys

sys.path.insert(0, "/opt/trn_rl_repo")

import numpy as np

import concourse.bass as bass
import concourse.mybir as mybir
import concourse.tile as tile
from concourse import bacc

F32 = mybir.dt.float32
BF16 = mybir.dt.bfloat16
AF = mybir.ActivationFunctionType

DIM = 1024      # model dim (contraction for projections)
CH = 512        # per-core channels (8 heads x 64)
P = 128

N_CORES = 8
B, T_FULL = 4, 4096

N_WARMUP = 30   # dummy matmuls to lift the HAM clock gate during input DMA


def build_nc(T=T_FULL):
    NTB = T // 512          # 512-token blocks
    nc = bacc.Bacc(None, target_bir_lowering=False, debug=False)

    xT = nc.declare_dram_parameter("xT", [DIM, T], BF16, isOutput=False)
    w1 = nc.declare_dram_parameter("w1", [DIM, 3 * CH], BF16, isOutput=False)
    w2 = nc.declare_dram_parameter("w2", [CH, DIM], BF16, isOutput=False)
    ec = nc.declare_dram_parameter("ec", [P, P], BF16, isOutput=False)
    y = nc.declare_dram_parameter("y", [T, DIM], BF16, isOutput=True)

    with tile.TileContext(nc) as tc:
        with tc.tile_pool(name="persist", bufs=1) as pp:
            ones_col = pp.tile([P, 1], BF16, name="ones_col", tag="ones_col")
            nc.vector.memset(ones_col[:, :], 1.0)

            # k columns of W1 first: the first k-matmul group only needs
            # these + the first x block (2 MB) instead of all of W1.
            w1k, w1v, w1q = [], [], []
            for ct in range(8):
                t_ = pp.tile([P, CH], BF16, name=f"w1k_{ct}", tag=f"w1k_{ct}")
                nc.sync.dma_start(out=t_[:, :], in_=w1[ct * P:(ct + 1) * P, CH:2 * CH])
                w1k.append(t_)
                w1v.append(pp.tile([P, CH], BF16, name=f"w1v_{ct}", tag=f"w1v_{ct}"))
                w1q.append(pp.tile([P, CH], BF16, name=f"w1q_{ct}", tag=f"w1q_{ct}"))

            qt = [
                [
                    pp.tile([P, 512], BF16, name=f"qt_{ib}_{j}", tag=f"qt_{ib}_{j}")
                    for j in range(4)
                ]
                for ib in range(NTB)
            ]

            kvt = [
                pp.tile([P, P], BF16, name=f"kvt_{j}", tag=f"kvt_{j}")
                for j in range(4)
            ]
            zt = pp.tile([1, CH], BF16, name="zt", tag="zt")

            w2t, Ms = [], []
            for j in range(4):
                w2t.append(pp.tile([P, DIM], BF16, name=f"w2_{j}", tag=f"w2_{j}"))
                Ms.append(pp.tile([P, DIM], BF16, name=f"Ms_{j}", tag=f"Ms_{j}"))

            Zb = [
                pp.tile([P, 8], BF16, name=f"Zb_{j}", tag=f"Zb_{j}")
                for j in range(4)
            ]
            ec_sb = pp.tile([P, P], BF16, name="ec_sb", tag="ec_sb")

            phase1(nc, tc, pp, T, NTB, xT, w1, w2, ec, ec_sb, w1k, w1v, w1q,
                   qt, kvt, zt, w2t, ones_col, Zb)
            phase2(nc, tc, pp, T, NTB, y, qt, kvt, w2t, Ms, Zb, ec_sb)

    nc.compile()
    return nc


def evict_kv_z(nc, kvps, zps, kvt, zt, Zb):
    """Evict the kv/z PSUM accumulators and stage Zb — split across
    vector (j0/j1), scalar (j2/j3 + z; it is nearly idle here) and
    gpsimd (SBUF memsets) so the M/den matmuls never wait on it."""
    for j in range(4):
        nc.vector.memset(kvt[j][:, :], 0.0)
    for j in range(2):
        nc.vector.tensor_copy(
            kvt[j][0:64, 0:64], kvps[0:64, j * P:j * P + 64]
        )
        nc.vector.tensor_copy(
            kvt[j][64:128, 64:128],
            kvps[64:128, j * P + 64:(j + 1) * P],
        )
    for j in range(2, 4):
        nc.scalar.copy(
            kvt[j][0:64, 0:64], kvps[0:64, j * P:j * P + 64]
        )
        nc.scalar.copy(
            kvt[j][64:128, 64:128],
            kvps[64:128, j * P + 64:(j + 1) * P],
        )
    nc.scalar.copy(zt[0:1, :], zps[0:1, :])
    for j in range(4):
        nc.gpsimd.memset(Zb[j][:, :], 0.0)
        nc.sync.dma_start(
            out=Zb[j][0:64, 2 * j:2 * j + 1],
            in_=zt[0:1, j * P:j * P + 64],
        )
        nc.sync.dma_start(
            out=Zb[j][64:128, 2 * j + 1:2 * j + 2],
            in_=zt[0:1, j * P + 64:(j + 1) * P],
        )


def phase1(nc, tc, pp, T, NTB, xT, w1, w2, ec, ec_sb, w1k, w1v, w1q, qt, kvt,
           zt, w2t, ones_col, Zb):
    with (
        tc.tile_pool(name="ph1_sb", bufs=3) as pa,
        tc.tile_pool(name="kv_sb", bufs=3) as kvp,
        tc.tile_pool(name="xload", bufs=24) as xp,
        tc.tile_pool(name="proj_ps", bufs=6, space="PSUM") as proj_ps,
        tc.tile_pool(name="hold_ps", bufs=1, space="PSUM") as hold_ps,
    ):
        # Warm the PE clock gate with dummy matmuls while input DMAs run.
        wu = pa.tile([P, 512], BF16, name="wu", tag="elu_m")
        nc.vector.memset(wu[:, :], 0.0)
        wups = proj_ps.tile([P, 512], F32, name="wups", tag="proj")
        for i in range(N_WARMUP):
            nc.tensor.matmul(
                wups[:, :], wu[:, 0:P], wu[:, :],
                start=True, stop=True, skip_group_check=True,
            )

        # PSUM accumulators held across all of phase 1 (one bank each).
        kvps = hold_ps.tile([P, 4 * P], F32, name="kvps", tag="kvps")
        zps = hold_ps.tile([1, CH], F32, name="zps", tag="zps")
        nc.vector.memset(kvps[:, :], 0.0)

        pending = []            # deferred z/kv matmuls (closures)

        def flush_pending():
            while pending:
                pending.pop(0)()

        for ib in range(NTB):
            xt = []
            for ct in range(8):
                t_ = xp.tile([P, 512], BF16, name=f"xt_{ib}_{ct}", tag="xt")
                nc.sync.dma_start(
                    out=t_[:, :], in_=xT[ct * P:(ct + 1) * P, ib * 512:(ib + 1) * 512]
                )
                xt.append(t_)
            if ib == 0:
                for ct in range(8):
                    nc.sync.dma_start(
                        out=w1v[ct][:, :], in_=w1[ct * P:(ct + 1) * P, 2 * CH:3 * CH]
                    )
                for ct in range(8):
                    nc.sync.dma_start(
                        out=w1q[ct][:, :], in_=w1[ct * P:(ct + 1) * P, 0:CH]
                    )

            # k/v projections (token-major) per 128-token chunk, with the
            # z/kv matmuls of the previous chunk interleaved after each
            # vps group so they never wait on the elu chain.  For the first
            # block, all four k-groups run before any v-group so the PE
            # covers the in-flight w1v/w1q weight DMAs.
            k_sbs = {}

            def k_group(t):
                tok = slice(t * P, (t + 1) * P)
                kps = proj_ps.tile([P, 512], F32, name=f"kps_{ib}_{t}", tag="proj")
                for ct in range(8):
                    nc.tensor.matmul(
                        kps[:, :], xt[ct][:, tok], w1k[ct][:, :],
                        start=(ct == 0), stop=(ct == 7),
                    )
                km = pa.tile([P, 512], BF16, name=f"km_{ib}_{t}", tag="elu_m")
                ke = pa.tile([P, 512], BF16, name=f"ke_{ib}_{t}", tag="elu_e")
                kr = pa.tile([P, 512], BF16, name=f"kr_{ib}_{t}", tag="elu_r")
                k_sb = kvp.tile([P, 512], BF16, name=f"k_{ib}_{t}", tag="k_sb",
                                bufs=6)
                nc.vector.tensor_scalar_min(km[:, :], kps[:, :], 0.0)
                nc.scalar.activation(ke[:, :], km[:, :], AF.Exp)
                nc.scalar.activation(kr[:, :], kps[:, :], AF.Relu)
                nc.vector.tensor_add(k_sb[:, :], ke[:, :], kr[:, :])
                k_sbs[t] = k_sb

            def v_group(t):
                tok = slice(t * P, (t + 1) * P)
                vps = proj_ps.tile([P, 512], F32, name=f"vps_{ib}_{t}", tag="proj")
                for ct in range(8):
                    nc.tensor.matmul(
                        vps[:, :], xt[ct][:, tok], w1v[ct][:, :],
                        start=(ct == 0), stop=(ct == 7),
                    )
                v_sb = kvp.tile([P, 512], BF16, name=f"v_{ib}_{t}", tag="v_sb")
                nc.vector.tensor_copy(v_sb[:, :], vps[:, :])
                return v_sb

            if ib == 0:
                for t in range(4):
                    k_group(t)
            for t in range(4):
                if ib != 0:
                    k_group(t)
                v_sb = v_group(t)
                k_sb = k_sbs[t]

                flush_pending()

                def defer(ib=ib, t=t, k_sb=k_sb, v_sb=v_sb):
                    first = (ib == 0 and t == 0)
                    last = (ib == NTB - 1 and t == 3)
                    # z += ones^T k   [1, 512]
                    nc.tensor.matmul(
                        zps[0:1, :], ones_col[:, :], k_sb[:, :],
                        start=first, stop=last, skip_group_check=True,
                    )
                    # kvT[j] += v_pair^T k_pair  [128,128] per head-pair; one
                    # accumulation region per j inside the pre-zeroed bank.
                    for j in range(4):
                        csl = slice(j * P, (j + 1) * P)
                        nc.tensor.matmul(
                            kvps[:, csl], v_sb[:, csl], k_sb[:, csl],
                            start=False, stop=(last and j == 3),
                            skip_group_check=True,
                        )
                pending.append(defer)

            # q projection (c-major) with elu+1, into persistent qt
            for j in range(4):
                qps = proj_ps.tile([P, 512], F32, name=f"qps_{ib}_{j}", tag="proj")
                for ct in range(8):
                    nc.tensor.matmul(
                        qps[:, :],
                        w1q[ct][:, j * P:(j + 1) * P],
                        xt[ct][:, :],
                        start=(ct == 0), stop=(ct == 7),
                    )
                if j == 0:
                    flush_pending()
                    if ib == NTB - 1:
                        # last kv/z matmuls just issued: evict accumulators
                        # while the remaining q groups keep the PE busy.
                        evict_kv_z(nc, kvps, zps, kvt, zt, Zb)
                qm = pa.tile([P, 512], BF16, name=f"qm_{ib}_{j}", tag="elu_m")
                qe = pa.tile([P, 512], BF16, name=f"qe_{ib}_{j}", tag="elu_e")
                qr = pa.tile([P, 512], BF16, name=f"qr_{ib}_{j}", tag="elu_r")
                nc.vector.tensor_scalar_min(qm[:, :], qps[:, :], 0.0)
                nc.scalar.activation(qe[:, :], qm[:, :], AF.Exp)
                nc.scalar.activation(qr[:, :], qps[:, :], AF.Relu)
                nc.vector.tensor_add(qt[ib][j][:, :], qe[:, :], qr[:, :])

            if ib == 0:
                # stage phase-2 constants off the critical path
                for j in range(4):
                    nc.sync.dma_start(
                        out=w2t[j][:, :], in_=w2[j * P:(j + 1) * P, :]
                    )
                nc.sync.dma_start(out=ec_sb[:, :], in_=ec[:, :])

        flush_pending()


def phase2(nc, tc, pp, T, NTB, y, qt, kvt, w2t, Ms, Zb, ec_sb):
    Es = [ec_sb[32 * j:32 * j + 8, :] for j in range(4)]

    with (
        tc.tile_pool(name="ph2_sb", bufs=2) as pb,
        tc.tile_pool(name="qsc_pool", bufs=8) as qp,
    ):
        rTs = [None] * NTB

        def den_stage(ib, dps_pool, d_bufs=1):
            dps = dps_pool.tile([8, 512], F32, name=f"dps_{ib}", tag="d",
                                bufs=d_bufs)
            for j in range(4):
                nc.tensor.matmul(
                    dps[:, :], Zb[j][:, :], qt[ib][j][:, :],
                    start=(j == 0), stop=(j == 3),
                )
            den_sb = pb.tile([8, 512], F32, name=f"den_{ib}", tag="den_sb")
            nc.vector.tensor_scalar_add(den_sb[:, :], dps[:, :], 1e-6)
            # partition-blocked reshape: [8 heads, 512 tok] ->
            # [128 = head*16 + tok//32, 32 = tok%32]  (linearized DMA)
            den_rs = pb.tile([P, 32], F32, name=f"drs_{ib}", tag="den_rs")
            nc.sync.dma_start(out=den_rs[:, :], in_=den_sb[:, :])
            rr = pb.tile([P, 32], BF16, name=f"rr_{ib}", tag="rr")
            with nc.allow_low_precision(reason="r is O(1e-5); bf16 matches op dtype"):
                nc.vector.reciprocal(rr[:, :], den_rs[:, :])
            rT = pb.tile([P, 512], BF16, name=f"rT_{ib}", tag="rT", bufs=6)
            for g in range(4):
                nc.sync.dma_start(out=rT[32 * g:32 * g + 8, :], in_=rr[:, :])
            rTs[ib] = rT

        qscs = [None] * NTB

        def qsc_stage(ib, dps_pool):
            qsc = []
            for j in range(4):
                bcp = dps_pool.tile([P, 512], F32, name=f"bcp_{ib}_{j}", tag="bc", bufs=4)
                nc.tensor.matmul(
                    bcp[:, :], Es[j][:, :], rTs[ib][32 * j:32 * j + 8, :],
                    start=True, stop=True, tile_position=(32 * j, 0),
                )
                qs = qp.tile([P, 512], BF16, name=f"qsc_{ib}_{j}", tag="qsc")
                nc.vector.tensor_mul(qs[:, :], qt[ib][j][:, :], bcp[:, :])
                qsc.append(qs)
            qscs[ib] = qsc

        def y_stage(ib, yps_pool):
            qsc = qscs[ib]
            for t in range(4):
                tok = slice(t * P, (t + 1) * P)
                row = (ib * 4 + t) * P
                last = (ib == NTB - 1 and t == 3)
                y_sb = pb.tile([P, DIM], BF16, name=f"y_{ib}_{t}", tag="y_sb",
                               bufs=4)
                for h in range(2):
                    hsl = slice(h * 512, (h + 1) * 512)
                    yp = yps_pool.tile([P, 512], F32, name=f"yps_{ib}_{t}_{h}",
                                       tag="y")
                    for j in range(4):
                        nc.tensor.matmul(
                            yp[:, :], qsc[j][:, tok], Ms[j][:, hsl],
                            start=(j == 0), stop=(j == 3),
                        )
                    if h == 0:
                        nc.vector.tensor_copy(y_sb[:, hsl], yp[:, :])
                        if last:
                            # store halves separately to shorten the tail
                            nc.sync.dma_start(
                                out=y[row:row + P, hsl], in_=y_sb[:, hsl]
                            )
                    else:
                        nc.scalar.copy(y_sb[:, hsl], yp[:, :])
                        if last:
                            nc.scalar.dma_start(
                                out=y[row:row + P, hsl], in_=y_sb[:, hsl]
                            )
                if not last:
                    # scalar is a HWDGE engine: issue the y store there to
                    # keep the sync queue free for the den->rT chain.
                    nc.scalar.dma_start(out=y[row:row + P, :], in_=y_sb[:, :])

        # Boundary: interleave the M matmuls with the first four den
        # stages so the PE rides through the scalar-side M evictions.
        with tc.tile_pool(name="m_ps", bufs=4, space="PSUM") as mps_pool:
            for j in range(4):
                for h in range(2):
                    hsl = slice(h * 512, (h + 1) * 512)
                    mps = mps_pool.tile([P, 512], F32, name=f"mps_{j}_{h}",
                                        tag="m")
                    nc.tensor.matmul(
                        mps[:, :], kvt[j][:, :], w2t[j][:, hsl],
                        start=True, stop=True,
                    )
                    # scalar: the vector queue is busy with the phase-1
                    # tail and the den->r chain here
                    nc.scalar.copy(Ms[j][:, hsl], mps[:, :])
                den_stage(j, mps_pool)

        with (
            tc.tile_pool(name="d_ps", bufs=2, space="PSUM") as dps_pool,
            tc.tile_pool(name="y_ps", bufs=3, space="PSUM") as yps_pool,
        ):
            for ib in range(4, NTB):
                qsc_stage(ib - 4, dps_pool)
                den_stage(ib, dps_pool)
                y_stage(ib - 4, yps_pool)
            for ib in range(NTB - 4, NTB):
                qsc_stage(ib, dps_pool)
                y_stage(ib, yps_pool)


_NC_CACHE = {}


def _get_nc(T=T_FULL):
    if T not in _NC_CACHE:
        _NC_CACHE[T] = build_nc(T)
    return _NC_CACHE[T]


def make_in_maps(x, W_qkv, W_out, b_out):
    import ml_dtypes

    bf16 = ml_dtypes.bfloat16
    x = np.asarray(x, dtype=np.float32)
    W_qkv = np.asarray(W_qkv, dtype=np.float32).astype(bf16)
    W_out = np.asarray(W_out, dtype=np.float32).astype(bf16)

    xTs = [np.ascontiguousarray(x[b].T.astype(bf16)) for b in range(B)]
    w1s, w2s = [], []
    for hg in range(2):
        cs = slice(hg * CH, (hg + 1) * CH)
        w1s.append(
            np.ascontiguousarray(
                np.concatenate(
                    [W_qkv[:, cs],
                     W_qkv[:, DIM + hg * CH:DIM + (hg + 1) * CH],
                     W_qkv[:, 2 * DIM + hg * CH:2 * DIM + (hg + 1) * CH]],
                    axis=1,
                )
            )
        )
        w2s.append(np.ascontiguousarray(W_out[cs, :]))

    ecm = make_ec().astype(bf16)
    in_maps = []
    for core in range(N_CORES):
        b, hg = core // 2, core % 2
        in_maps.append({"xT": xTs[b], "w1": w1s[hg], "w2": w2s[hg], "ec": ecm})
    return in_maps


def make_ec():
    """E selector staged per PE row-group: rows 32j..32j+8 hold E_j with
    E_j[h, p] = 1 iff head-of-partition-p-in-tile-j == h."""
    ecm = np.zeros((P, P), dtype=np.float32)
    for j in range(4):
        ecm[32 * j + 2 * j, 0:64] = 1.0
        ecm[32 * j + 2 * j + 1, 64:128] = 1.0
    return ecm


def kernel(x, W_qkv, W_out, b_out):
    from concourse.bass_utils import run_bass_kernel_spmd

    nc = _get_nc(T_FULL)
    in_maps = make_in_maps(x, W_qkv, W_out, b_out)
    res = run_bass_kernel_spmd(nc, in_maps, core_ids=list(range(N_CORES))).results
    bo = np.asarray(b_out, dtype=np.float32)
    out = np.empty((B, T_FULL, DIM), dtype=np.float32)
    for b in range(B):
        out[b] = (res[2 * b]["y"].astype(np.float32)
                  + res[2 * b + 1]["y"].astype(np.float32) + bo)
    return out



# revision 8
# speedup vs baseline: 1.2139x; 1.2139x over previous
"""Linear attention kernel for 8 Trainium2 NeuronCores.

Sharding: core = 2*b + hg  (b in 0..3 batches, hg in 0..1 head-groups of 8 heads).
Fully data-parallel — no collectives; host sums the two head-group partials per
batch (f32) and adds the bias.

Per-core math (T=4096 tokens, CH=512 = 8 heads x 64, DIM=1024):
  Phase 1 (per 512-token block): k,v = x @ Wk, x @ Wv token-major; elu+1 on k;
    qT = (x @ Wq)^T c-major with elu+1 (persisted for phase 2);
    kvT += v^T k per head-pair (diagonal 64-blocks), z += ones^T k, in PSUM.
    The z/kv matmuls are deferred one token-chunk so the in-order PE never
    waits on the elu chain.
  Boundary: evict kvT/z; M = kvT^T @ W2 interleaved with the first den
    stages; Zb = block-diag expansion of z.
  Phase 2 (per 512-token block, software-pipelined 4 blocks deep so the
    in-order PE never waits on the r chain): den = Zb^T qT [8, 512];
    r = 1/(den+1e-6) computed in a [128,32] partition-blocked layout (DMA
    reshape both ways; 16x fewer elems per DVE lane than [8,512]);
    rbc = E^T r broadcast matmul; qsc = qT * rbc;
    y = qsc^T @ M, written bf16 (host accumulates in f32).

  A short burst of dummy matmuls at the start lifts the PE HAM clock gate
  (1.2 -> 2.4 GHz) while the first input DMAs are still in flight.
"""

import sys

sys.path.insert(0, "/opt/trn_rl_repo")

import numpy as np

import concourse.bass as bass
import concourse.mybir as mybir
import concourse.tile as tile
from concourse import bacc

F32 = mybir.dt.float32
BF16 = mybir.dt.bfloat16
F8 = mybir.dt.float8e4
AF = mybir.ActivationFunctionType
DR = mybir.MatmulPerfMode.DoubleRow

DIM = 1024      # model dim (contraction for projections)
CH = 512        # per-core channels (8 heads x 64)
P = 128

N_CORES = 8
B, T_FULL = 4, 4096

N_WARMUP = 30   # dummy matmuls to lift the HAM clock gate during input DMA

# k/q projections run in fp8e4 DoubleRow (2x PE rate).  W1 is scaled by
# WS host-side so its +-1/32 values clear the fp8 min-normal (2^-6); the
# elu descales via the activation scale.  v stays bf16: attention output
# is a weighted average of v, so v's quantization noise survives to the
# output undamped (k/q noise largely cancels in the num/den ratio).
WS = 16.0
WS_INV = 1.0 / WS


def build_nc(T=T_FULL):
    NTB = T // 512          # 512-token blocks
    nc = bacc.Bacc(None, target_bir_lowering=False, debug=False)

    # fp8 operands for k/q (row-pair interleaved for DoubleRow); bf16 for v.
    xT8 = nc.declare_dram_parameter("xT8", [4 * P, 2, T], F8, isOutput=False)
    xT = nc.declare_dram_parameter("xT", [DIM, T], BF16, isOutput=False)
    w18 = nc.declare_dram_parameter("w18", [4 * P, 2, 2 * CH], F8, isOutput=False)
    w1v_d = nc.declare_dram_parameter("w1v", [DIM, CH], BF16, isOutput=False)
    w2 = nc.declare_dram_parameter("w2", [CH, DIM], BF16, isOutput=False)
    ec = nc.declare_dram_parameter("ec", [P, P], BF16, isOutput=False)
    y = nc.declare_dram_parameter("y", [T, DIM], BF16, isOutput=True)

    with tile.TileContext(nc) as tc:
        with tc.tile_pool(name="persist", bufs=1) as pp:
            ones_col = pp.tile([P, 1], BF16, name="ones_col", tag="ones_col")
            nc.vector.memset(ones_col[:, :], 1.0)

            # k columns of W1 first: the first k-matmul group only needs
            # these + the first x block instead of all of W1.
            w1k, w1v, w1q = [], [], []
            for ct in range(4):
                t_ = pp.tile([P, 2, CH], F8, name=f"w1k_{ct}", tag=f"w1k_{ct}")
                nc.sync.dma_start(
                    out=t_[:, :, :], in_=w18[ct * P:(ct + 1) * P, :, 0:CH]
                )
                w1k.append(t_)
                w1q.append(pp.tile([P, 2, CH], F8, name=f"w1q_{ct}", tag=f"w1q_{ct}"))
            for ct in range(8):
                w1v.append(pp.tile([P, CH], BF16, name=f"w1v_{ct}", tag=f"w1v_{ct}"))

            qt = [
                [
                    pp.tile([P, 512], BF16, name=f"qt_{ib}_{j}", tag=f"qt_{ib}_{j}")
                    for j in range(4)
                ]
                for ib in range(NTB)
            ]

            kvt = [
                pp.tile([P, P], BF16, name=f"kvt_{j}", tag=f"kvt_{j}")
                for j in range(4)
            ]
            zt = pp.tile([1, CH], BF16, name="zt", tag="zt")

            w2t, Ms = [], []
            for j in range(4):
                w2t.append(pp.tile([P, DIM], BF16, name=f"w2_{j}", tag=f"w2_{j}"))
                Ms.append(pp.tile([P, DIM], BF16, name=f"Ms_{j}", tag=f"Ms_{j}"))

            Zb = [
                pp.tile([P, 8], BF16, name=f"Zb_{j}", tag=f"Zb_{j}")
                for j in range(4)
            ]
            ec_sb = pp.tile([P, P], BF16, name="ec_sb", tag="ec_sb")

            phase1(nc, tc, pp, T, NTB, xT8, xT, w18, w1v_d, w2, ec, ec_sb,
                   w1k, w1v, w1q, qt, kvt, zt, w2t, ones_col, Zb)
            phase2(nc, tc, pp, T, NTB, y, qt, kvt, w2t, Ms, Zb, ec_sb)

    nc.compile()
    return nc


def evict_kv_z(nc, kvps, zps, kvt, zt, Zb):
    """Evict the kv/z PSUM accumulators and stage Zb — split across
    vector (j0/j1), scalar (j2/j3 + z; it is nearly idle here) and
    gpsimd (SBUF memsets) so the M/den matmuls never wait on it."""
    for j in range(4):
        nc.vector.memset(kvt[j][:, :], 0.0)
    for j in range(2):
        nc.vector.tensor_copy(
            kvt[j][0:64, 0:64], kvps[0:64, j * P:j * P + 64]
        )
        nc.vector.tensor_copy(
            kvt[j][64:128, 64:128],
            kvps[64:128, j * P + 64:(j + 1) * P],
        )
    for j in range(2, 4):
        nc.scalar.copy(
            kvt[j][0:64, 0:64], kvps[0:64, j * P:j * P + 64]
        )
        nc.scalar.copy(
            kvt[j][64:128, 64:128],
            kvps[64:128, j * P + 64:(j + 1) * P],
        )
    nc.scalar.copy(zt[0:1, :], zps[0:1, :])
    for j in range(4):
        nc.gpsimd.memset(Zb[j][:, :], 0.0)
        nc.sync.dma_start(
            out=Zb[j][0:64, 2 * j:2 * j + 1],
            in_=zt[0:1, j * P:j * P + 64],
        )
        nc.sync.dma_start(
            out=Zb[j][64:128, 2 * j + 1:2 * j + 2],
            in_=zt[0:1, j * P + 64:(j + 1) * P],
        )


def phase1(nc, tc, pp, T, NTB, xT8, xT, w18, w1v_d, w2, ec, ec_sb, w1k, w1v,
           w1q, qt, kvt, zt, w2t, ones_col, Zb):
    with (
        tc.tile_pool(name="ph1_sb", bufs=3) as pa,
        tc.tile_pool(name="kv_sb", bufs=3) as kvp,
        tc.tile_pool(name="xload", bufs=24) as xp,
        tc.tile_pool(name="proj_ps", bufs=6, space="PSUM") as proj_ps,
        tc.tile_pool(name="hold_ps", bufs=1, space="PSUM") as hold_ps,
    ):
        # Warm the PE clock gate with dummy matmuls while input DMAs run.
        wu = pa.tile([P, 512], BF16, name="wu", tag="elu_m")
        nc.vector.memset(wu[:, :], 0.0)
        wups = proj_ps.tile([P, 512], F32, name="wups", tag="proj")
        for i in range(N_WARMUP):
            nc.tensor.matmul(
                wups[:, :], wu[:, 0:P], wu[:, :],
                start=True, stop=True, skip_group_check=True,
            )

        # PSUM accumulators held across all of phase 1 (one bank each).
        kvps = hold_ps.tile([P, 4 * P], F32, name="kvps", tag="kvps")
        zps = hold_ps.tile([1, CH], F32, name="zps", tag="zps")
        nc.vector.memset(kvps[:, :], 0.0)

        pending = []            # deferred z/kv matmuls (closures)

        def flush_pending():
            while pending:
                pending.pop(0)()

        for ib in range(NTB):
            xt8 = []
            for ct in range(4):
                t_ = xp.tile([P, 2, 512], F8, name=f"xt8_{ib}_{ct}", tag="xt8")
                nc.sync.dma_start(
                    out=t_[:, :, :],
                    in_=xT8[ct * P:(ct + 1) * P, :, ib * 512:(ib + 1) * 512],
                )
                xt8.append(t_)
            xt = []
            for ct in range(8):
                t_ = xp.tile([P, 512], BF16, name=f"xt_{ib}_{ct}", tag="xt")
                nc.sync.dma_start(
                    out=t_[:, :], in_=xT[ct * P:(ct + 1) * P, ib * 512:(ib + 1) * 512]
                )
                xt.append(t_)
            if ib == 0:
                for ct in range(8):
                    nc.sync.dma_start(
                        out=w1v[ct][:, :], in_=w1v_d[ct * P:(ct + 1) * P, :]
                    )
                for ct in range(4):
                    nc.sync.dma_start(
                        out=w1q[ct][:, :, :],
                        in_=w18[ct * P:(ct + 1) * P, :, CH:2 * CH],
                    )

            # k/v projections (token-major) per 128-token chunk, with the
            # z/kv matmuls of the previous chunk interleaved after each
            # vps group so they never wait on the elu chain.  For the first
            # block, all four k-groups run before any v-group so the PE
            # covers the in-flight w1v/w1q weight DMAs.
            k_sbs = {}

            def k_group(t):
                tok = slice(t * P, (t + 1) * P)
                kps = proj_ps.tile([P, 512], F32, name=f"kps_{ib}_{t}", tag="proj")
                for ct in range(4):
                    nc.tensor.matmul(
                        kps[:, :], xt8[ct][:, :, tok], w1k[ct][:, :, :],
                        start=(ct == 0), stop=(ct == 3), perf_mode=DR,
                    )
                km = pa.tile([P, 512], BF16, name=f"km_{ib}_{t}", tag="elu_m")
                ke = pa.tile([P, 512], BF16, name=f"ke_{ib}_{t}", tag="elu_e")
                kr = pa.tile([P, 512], BF16, name=f"kr_{ib}_{t}", tag="elu_r")
                k_sb = kvp.tile([P, 512], BF16, name=f"k_{ib}_{t}", tag="k_sb",
                                bufs=6)
                nc.vector.tensor_scalar_min(km[:, :], kps[:, :], 0.0)
                nc.scalar.activation(ke[:, :], km[:, :], AF.Exp, scale=WS_INV)
                nc.scalar.activation(kr[:, :], kps[:, :], AF.Relu, scale=WS_INV)
                nc.vector.tensor_add(k_sb[:, :], ke[:, :], kr[:, :])
                k_sbs[t] = k_sb

            def v_group(t):
                tok = slice(t * P, (t + 1) * P)
                vps = proj_ps.tile([P, 512], F32, name=f"vps_{ib}_{t}", tag="proj")
                for ct in range(8):
                    nc.tensor.matmul(
                        vps[:, :], xt[ct][:, tok], w1v[ct][:, :],
                        start=(ct == 0), stop=(ct == 7),
                    )
                v_sb = kvp.tile([P, 512], BF16, name=f"v_{ib}_{t}", tag="v_sb")
                nc.vector.tensor_copy(v_sb[:, :], vps[:, :])
                return v_sb

            if ib == 0:
                for t in range(4):
                    k_group(t)
            for t in range(4):
                if ib != 0:
                    k_group(t)
                v_sb = v_group(t)
                k_sb = k_sbs[t]

                flush_pending()

                def defer(ib=ib, t=t, k_sb=k_sb, v_sb=v_sb):
                    first = (ib == 0 and t == 0)
                    last = (ib == NTB - 1 and t == 3)
                    # z += ones^T k   [1, 512]
                    nc.tensor.matmul(
                        zps[0:1, :], ones_col[:, :], k_sb[:, :],
                        start=first, stop=last, skip_group_check=True,
                    )
                    # kvT[j] += v_pair^T k_pair  [128,128] per head-pair; one
                    # accumulation region per j inside the pre-zeroed bank.
                    for j in range(4):
                        csl = slice(j * P, (j + 1) * P)
                        nc.tensor.matmul(
                            kvps[:, csl], v_sb[:, csl], k_sb[:, csl],
                            start=False, stop=(last and j == 3),
                            skip_group_check=True,
                        )
                pending.append(defer)

            # q projection (c-major) with elu+1, into persistent qt
            for j in range(4):
                qps = proj_ps.tile([P, 512], F32, name=f"qps_{ib}_{j}", tag="proj")
                for ct in range(4):
                    nc.tensor.matmul(
                        qps[:, :],
                        w1q[ct][:, :, j * P:(j + 1) * P],
                        xt8[ct][:, :, :],
                        start=(ct == 0), stop=(ct == 3), perf_mode=DR,
                    )
                if j == 0:
                    flush_pending()
                    if ib == NTB - 1:
                        # last kv/z matmuls just issued: evict accumulators
                        # while the remaining q groups keep the PE busy.
                        evict_kv_z(nc, kvps, zps, kvt, zt, Zb)
                qm = pa.tile([P, 512], BF16, name=f"qm_{ib}_{j}", tag="elu_m")
                qe = pa.tile([P, 512], BF16, name=f"qe_{ib}_{j}", tag="elu_e")
                qr = pa.tile([P, 512], BF16, name=f"qr_{ib}_{j}", tag="elu_r")
                nc.vector.tensor_scalar_min(qm[:, :], qps[:, :], 0.0)
                nc.scalar.activation(qe[:, :], qm[:, :], AF.Exp, scale=WS_INV)
                nc.scalar.activation(qr[:, :], qps[:, :], AF.Relu, scale=WS_INV)
                nc.vector.tensor_add(qt[ib][j][:, :], qe[:, :], qr[:, :])

            if ib == 0:
                # stage phase-2 constants off the critical path
                for j in range(4):
                    nc.sync.dma_start(
                        out=w2t[j][:, :], in_=w2[j * P:(j + 1) * P, :]
                    )
                nc.sync.dma_start(out=ec_sb[:, :], in_=ec[:, :])

        flush_pending()


def phase2(nc, tc, pp, T, NTB, y, qt, kvt, w2t, Ms, Zb, ec_sb):
    Es = [ec_sb[32 * j:32 * j + 8, :] for j in range(4)]

    with (
        tc.tile_pool(name="ph2_sb", bufs=2) as pb,
        tc.tile_pool(name="qsc_pool", bufs=8) as qp,
    ):
        rTs = [None] * NTB

        def den_stage(ib, dps_pool, d_bufs=1):
            dps = dps_pool.tile([8, 512], F32, name=f"dps_{ib}", tag="d",
                                bufs=d_bufs)
            for j in range(4):
                nc.tensor.matmul(
                    dps[:, :], Zb[j][:, :], qt[ib][j][:, :],
                    start=(j == 0), stop=(j == 3),
                )
            den_sb = pb.tile([8, 512], F32, name=f"den_{ib}", tag="den_sb")
            nc.vector.tensor_scalar_add(den_sb[:, :], dps[:, :], 1e-6)
            # partition-blocked reshape: [8 heads, 512 tok] ->
            # [128 = head*16 + tok//32, 32 = tok%32]  (linearized DMA)
            den_rs = pb.tile([P, 32], F32, name=f"drs_{ib}", tag="den_rs")
            nc.sync.dma_start(out=den_rs[:, :], in_=den_sb[:, :])
            rr = pb.tile([P, 32], BF16, name=f"rr_{ib}", tag="rr")
            with nc.allow_low_precision(reason="r is O(1e-5); bf16 matches op dtype"):
                nc.vector.reciprocal(rr[:, :], den_rs[:, :])
            rT = pb.tile([P, 512], BF16, name=f"rT_{ib}", tag="rT", bufs=6)
            for g in range(4):
                nc.sync.dma_start(out=rT[32 * g:32 * g + 8, :], in_=rr[:, :])
            rTs[ib] = rT

        qscs = [None] * NTB

        def qsc_stage(ib, dps_pool):
            qsc = []
            for j in range(4):
                bcp = dps_pool.tile([P, 512], F32, name=f"bcp_{ib}_{j}", tag="bc", bufs=4)
                nc.tensor.matmul(
                    bcp[:, :], Es[j][:, :], rTs[ib][32 * j:32 * j + 8, :],
                    start=True, stop=True, tile_position=(32 * j, 0),
                )
                qs = qp.tile([P, 512], BF16, name=f"qsc_{ib}_{j}", tag="qsc")
                nc.vector.tensor_mul(qs[:, :], qt[ib][j][:, :], bcp[:, :])
                qsc.append(qs)
            qscs[ib] = qsc

        def y_stage(ib, yps_pool):
            qsc = qscs[ib]
            for t in range(4):
                tok = slice(t * P, (t + 1) * P)
                row = (ib * 4 + t) * P
                last = (ib == NTB - 1 and t == 3)
                y_sb = pb.tile([P, DIM], BF16, name=f"y_{ib}_{t}", tag="y_sb",
                               bufs=4)
                for h in range(2):
                    hsl = slice(h * 512, (h + 1) * 512)
                    yp = yps_pool.tile([P, 512], F32, name=f"yps_{ib}_{t}_{h}",
                                       tag="y")
                    for j in range(4):
                        nc.tensor.matmul(
                            yp[:, :], qsc[j][:, tok], Ms[j][:, hsl],
                            start=(j == 0), stop=(j == 3),
                        )
                    if h == 0:
                        nc.vector.tensor_copy(y_sb[:, hsl], yp[:, :])
                        if last:
                            # store halves separately to shorten the tail
                            nc.sync.dma_start(
                                out=y[row:row + P, hsl], in_=y_sb[:, hsl]
                            )
                    else:
                        nc.scalar.copy(y_sb[:, hsl], yp[:, :])
                        if last:
                            nc.scalar.dma_start(
                                out=y[row:row + P, hsl], in_=y_sb[:, hsl]
                            )
                if not last:
                    # scalar is a HWDGE engine: issue the y store there to
                    # keep the sync queue free for the den->rT chain.
                    nc.scalar.dma_start(out=y[row:row + P, :], in_=y_sb[:, :])

        # Boundary: interleave the M matmuls with the first four den
        # stages so the PE rides through the scalar-side M evictions.
        with tc.tile_pool(name="m_ps", bufs=4, space="PSUM") as mps_pool:
            for j in range(4):
                for h in range(2):
                    hsl = slice(h * 512, (h + 1) * 512)
                    mps = mps_pool.tile([P, 512], F32, name=f"mps_{j}_{h}",
                                        tag="m")
                    nc.tensor.matmul(
                        mps[:, :], kvt[j][:, :], w2t[j][:, hsl],
                        start=True, stop=True,
                    )
                    # scalar: the vector queue is busy with the phase-1
                    # tail and the den->r chain here
                    nc.scalar.copy(Ms[j][:, hsl], mps[:, :])
                den_stage(j, mps_pool)

        with (
            tc.tile_pool(name="d_ps", bufs=2, space="PSUM") as dps_pool,
            tc.tile_pool(name="y_ps", bufs=3, space="PSUM") as yps_pool,
        ):
            for ib in range(4, NTB):
                qsc_stage(ib - 4, dps_pool)
                den_stage(ib, dps_pool)
                y_stage(ib - 4, yps_pool)
            for ib in range(NTB - 4, NTB):
                qsc_stage(ib, dps_pool)
                y_stage(ib, yps_pool)


_NC_CACHE = {}


def _get_nc(T=T_FULL):
    if T not in _NC_CACHE:
        _NC_CACHE[T] = build_nc(T)
    return _NC_CACHE[T]


def _dr_interleave(a):
    """[DIM, F] -> [DIM//2, 2, F] with row d = ct*256 + i*128 + p at
    [ct*128 + p, i, :] — the row-pair layout DoubleRow matmuls consume."""
    F = a.shape[1]
    return np.ascontiguousarray(
        a.reshape(4, 2, P, F).transpose(0, 2, 1, 3).reshape(4 * P, 2, F)
    )


def make_in_maps(x, W_qkv, W_out, b_out):
    import ml_dtypes

    bf16 = ml_dtypes.bfloat16
    f8 = ml_dtypes.float8_e4m3
    x = np.asarray(x, dtype=np.float32)
    W_qkv = np.asarray(W_qkv, dtype=np.float32)
    W_out = np.asarray(W_out, dtype=np.float32).astype(bf16)

    xTs, xT8s = [], []
    for b in range(B):
        xt = np.ascontiguousarray(x[b].T)
        xTs.append(xt.astype(bf16))
        xT8s.append(_dr_interleave(xt.astype(f8)))
    w18s, w1vs, w2s = [], [], []
    for hg in range(2):
        cs = slice(hg * CH, (hg + 1) * CH)
        Wq = W_qkv[:, cs]
        Wk = W_qkv[:, DIM + hg * CH:DIM + (hg + 1) * CH]
        Wv = W_qkv[:, 2 * DIM + hg * CH:2 * DIM + (hg + 1) * CH]
        w18s.append(
            _dr_interleave(
                np.concatenate([Wk, Wq], axis=1).astype(np.float32) * WS
            ).astype(f8)
        )
        w1vs.append(np.ascontiguousarray(Wv.astype(bf16)))
        w2s.append(np.ascontiguousarray(W_out[cs, :]))

    ecm = make_ec().astype(bf16)
    in_maps = []
    for core in range(N_CORES):
        b, hg = core // 2, core % 2
        in_maps.append({
            "xT8": xT8s[b], "xT": xTs[b], "w18": w18s[hg],
            "w1v": w1vs[hg], "w2": w2s[hg], "ec": ecm,
        })
    return in_maps


def make_ec():
    """E selector staged per PE row-group: rows 32j..32j+8 hold E_j with
    E_j[h, p] = 1 iff head-of-partition-p-in-tile-j == h."""
    ecm = np.zeros((P, P), dtype=np.float32)
    for j in range(4):
        ecm[32 * j + 2 * j, 0:64] = 1.0
        ecm[32 * j + 2 * j + 1, 64:128] = 1.0
    return ecm


def kernel(x, W_qkv, W_out, b_out):
    from concourse.bass_utils import run_bass_kernel_spmd

    nc = _get_nc(T_FULL)
    in_maps = make_in_maps(x, W_qkv, W_out, b_out)
    res = run_bass_kernel_spmd(nc, in_maps, core_ids=list(range(N_CORES))).results
    bo = np.asarray(b_out, dtype=np.float32)
    out = np.empty((B, T_FULL, DIM), dtype=np.float32)
    for b in range(B):
        out[b] = (res[2 * b]["y"].astype(np.float32)
                  + res[2 * b + 1]["y"].astype(np.float32) + bo)
    return out

